# revision 16
# baseline (speedup 1.0000x reference)
"""Multi-head self-attention (RoPE + causal softmax) Bass kernel for TRN2.

Problem: B=2, H=16, S=2048, D_HEAD=64, fp32 I/O.
Sharding: 32 head-instances (B*H) split 4-per-core across 8 NeuronCores;
no cross-device communication.

v3 design (per core, 4 heads = 2 stacked pairs):
  - Q,K ship host-pre-transposed bf16 with their pair-swapped sign-folded
    shuffles packed alongside (kks/qqs = [128, 2S]); cos|sin packed as one
    [128, 2S] table tensor; V (+ones column) as one 4-head tensor.  All
    loads via the two HWDGE rings (sync + scalar) as a few big DMAs.
  - RoPE on DVE: rot = k*cosT + kshuf*sinT, all-bf16 2x mode, emitted per
    1024-col half into separate half tiles so score matmuls start as soon
    as the first halves are rotated.
  - Scores per 128-row k-tile into [128, 2x512] PSUM (head A | head B),
    causally trimmed; head pair shares the PE via row-group concurrency.
  - exp(s/8): ScalarE exact exp for diagonal tiles + most others; a
    configurable number of old (large-context) k-tiles per chunk use a
    bf16 Schraudolph bit-trick on DVE (i16 = round(s*A+B) viewed as bf16).
  - attn@[V|1] accumulates outT [65, 2x512] over k-tiles; row 64 is the
    softmax denominator.
  - Normalization stays transposed (no PE transposes): copy PSUM->SBUF,
    GpSimd divide -> reciprocal row, PE ones-matmul broadcasts it across
    partitions, DVE multiply -> bf16 [64, q] output, DMA'd transposed;
    host un-transposes.  The PE broadcast is deferred one chunk so the
    in-order PE queue never waits on the divide.
  - Continuous PE warmup bridges the load/RoPE phase.
"""

import math

import numpy as np
import ml_dtypes

import concourse.bass as bass
import concourse.tile as tile
from concourse import bacc, mybir
from concourse.bass_utils import run_bass_kernel_spmd

F32 = mybir.dt.float32
BF16 = mybir.dt.bfloat16
I16 = mybir.dt.int16
EXP = mybir.ActivationFunctionType.Exp
MULT = mybir.AluOpType.mult
ADD = mybir.AluOpType.add
DIV = mybir.AluOpType.divide

B, H, S_FULL, DH = 2, 16, 2048, 64
N_CORES = 8
HEADS_PER_CORE = (B * H) // N_CORES  # 4

# Schraudolph fast-exp constants for bf16 (computing exp(s/8)):
# i16 = round(s * FE_A + FE_B); bits(i16) viewed as bf16 ~= exp(s/8).
FE_A = 128.0 / math.log(2.0) * 0.125
FE_B = 127.0 * 128.0 - 7.38

# Number of oldest non-diagonal k-tiles per chunk whose exp runs on DVE
# via the bit-trick (large-context rows only; diagonal tiles and chunk 0
# always use exact ScalarE exp).
EXP_DVE = 2
WARMUP_MMS = 12


# ---------------------------------------------------------------- device IR


def build_nc(n_heads=HEADS_PER_CORE, S=S_FULL, chunk=512, num_devices=N_CORES,
             exp_dve=None, warmup=None):
    NT = S // 128
    npairs = n_heads // 2

    nc = bacc.Bacc(
        "TRN2", target_bir_lowering=False, debug=False, num_devices=num_devices
    )

    qqs = nc.dram_tensor("qqs", [npairs, 128, 2 * S], BF16, kind="ExternalInput").ap()
    kks = nc.dram_tensor("kks", [npairs, 128, 2 * S], BF16, kind="ExternalInput").ap()
    vx = nc.dram_tensor("vx", [n_heads, 128, NT * 65], BF16, kind="ExternalInput").ap()
    tt = nc.dram_tensor("tt", [128, 2 * S], BF16, kind="ExternalInput").ap()
    tri = nc.dram_tensor("tri", [128, 256], BF16, kind="ExternalInput").ap()
    ident = nc.dram_tensor("ident", [65, 65], F32, kind="ExternalInput").ap()
    o = nc.dram_tensor("o", [n_heads, 128, (S // 128) * DH], BF16,
                       kind="ExternalOutput").ap()

    with tile.TileContext(nc) as tc:
        _body(nc, tc, qqs, kks, vx, tt, tri, ident, o,
              n_heads=n_heads, S=S, chunk=chunk,
              exp_dve=EXP_DVE if exp_dve is None else exp_dve,
              warmup=WARMUP_MMS if warmup is None else warmup)

    nc.compile()
    return nc


def _body(nc, tc, qqs, kks, vx, tt, tri, ident, o, *, n_heads, S, chunk,
          exp_dve, warmup):
    from contextlib import ExitStack

    assert chunk == 512
    NT = S // 128
    npairs = n_heads // 2
    nchunks = S // chunk
    kpc = chunk // 128
    HSZ = S // 2  # rope half size

    with ExitStack() as ctx:
        cpool = ctx.enter_context(tc.tile_pool(name="const", bufs=1))
        raw = ctx.enter_context(tc.tile_pool(name="raw", bufs=1))
        rot = ctx.enter_context(tc.tile_pool(name="rot", bufs=1))
        prep = ctx.enter_context(tc.tile_pool(name="prep", bufs=2))
        expp = ctx.enter_context(tc.tile_pool(name="expp", bufs=3))
        sop = ctx.enter_context(tc.tile_pool(name="sop", bufs=2))
        rcp = ctx.enter_context(tc.tile_pool(name="rcp", bufs=4))
        obuf = ctx.enter_context(tc.tile_pool(name="obuf", bufs=1))
        ps_s = ctx.enter_context(tc.tile_pool(name="ps_s", bufs=2, space="PSUM"))
        ps_o = ctx.enter_context(tc.tile_pool(name="ps_o", bufs=1, space="PSUM"))
        ps_t = ctx.enter_context(tc.tile_pool(name="ps_t", bufs=2, space="PSUM"))

        # ---- warmup seed + ACT exp-table preload
        wt = cpool.tile([128, 512], BF16, tag="wt")
        nc.vector.memset(wt[:], 0.25)
        id_t = cpool.tile([65, 65], F32, tag="id")
        dme = cpool.tile([128, 8], BF16, tag="dme")
        nc.scalar.activation(dme[:], wt[:, 0:8], EXP, scale=0.125)

        # ---- inputs: sync ring carries pair-0 (critical path), scalar ring
        # carries tables/V/pair-1.  Few big DMAs; no SWDGE.
        kqr = []  # (kks_tile, qqs_tile) per pair
        for p in range(npairs):
            kt_ = raw.tile([128, 2 * S], BF16, tag=f"kk{p}", name=f"kk{p}")
            qt_ = raw.tile([128, 2 * S], BF16, tag=f"qq{p}", name=f"qq{p}")
            kqr.append((kt_, qt_))
        tt_t = cpool.tile([128, 2 * S], BF16, tag="tt")
        vall = cpool.tile([128, n_heads * NT * 65], BF16, tag="vall")
        tri_t = cpool.tile([128, 256], BF16, tag="tri")

        nc.sync.dma_start(kqr[0][0][:], kks[0])
        nc.sync.dma_start(kqr[0][1][:], qqs[0])
        nc.sync.dma_start(tri_t[:], tri[:])
        nc.sync.dma_start(id_t[:], ident[:])
        nc.scalar.dma_start(tt_t[:], tt[:])
        for h in range(n_heads):
            nc.scalar.dma_start(
                vall[:, h * NT * 65:(h + 1) * NT * 65], vx[h]
            )
        for p in range(1, npairs):
            nc.scalar.dma_start(kqr[p][0][:], kks[p])
            nc.scalar.dma_start(kqr[p][1][:], qqs[p])

        # ---- PE warmup bridge
        s_d = ps_s.tile([128, 1024], F32, tag="s")
        for _ in range(warmup):
            nc.tensor.matmul(s_d[:, 0:512], wt[0:64, 0:128], wt[0:64, 0:512],
                             start=True, stop=True)

        # ---- RoPE into per-half tiles: rot = nat*cos + shuf*sin (bf16 2x)
        cosv = tt_t[:, 0:S]
        sinv = tt_t[:, S:2 * S]
        kT = [[rot.tile([128, HSZ], BF16, tag=f"kT{p}{h}", name=f"kT{p}{h}")
               for h in range(2)] for p in range(npairs)]
        qT = [[rot.tile([128, HSZ], BF16, tag=f"qT{p}{h}", name=f"qT{p}{h}")
               for h in range(2)] for p in range(npairs)]
        chains = []
        for pr in range(npairs):
            chains.append((kqr[pr][0], kT[pr]))
            chains.append((kqr[pr][1], qT[pr]))
        for src, dsts in chains:
            for h in range(2):
                c0, c1 = h * HSZ, (h + 1) * HSZ
                t1 = prep.tile([128, HSZ], BF16, tag="t1")
                t2 = prep.tile([128, HSZ], BF16, tag="t2")
                nc.vector.tensor_mul(t1[:], src[:, c0:c1], cosv[:, c0:c1])
                nc.vector.tensor_mul(t2[:], src[:, S + c0:S + c1],
                                     sinv[:, c0:c1])
                nc.vector.tensor_add(dsts[h][:], t1[:], t2[:])

        def rslice(halves, base, lo, hi):
            # slice [lo:hi) columns out of the two half tiles, rows base:base+64
            h = lo // HSZ
            assert hi <= (h + 1) * HSZ
            return halves[h][base:base + 64, lo - h * HSZ:hi - h * HSZ]

        obs = [obuf.tile([128, NT * DH], BF16, tag=f"ob{h}", name=f"ob{h}")
               for h in range(n_heads)]

        # ---- main loop
        pending_norm = []

        def flush_norm():
            while pending_norm:
                pending_norm.pop(0)()

        for pr in range(npairs):
            hA, hB = 2 * pr, 2 * pr + 1
            v3A = vall[:, hA * NT * 65:(hA + 1) * NT * 65].rearrange(
                "p (t j) -> p t j", j=65)
            v3B = vall[:, hB * NT * 65:(hB + 1) * NT * 65].rearrange(
                "p (t j) -> p t j", j=65)
            for qc in range(nchunks):
                q0 = qc * chunk
                ktmax = (qc + 1) * kpc
                ndiag = qc * kpc
                out_t = ps_o.tile([65, 1024], F32, tag="out")

                stage = []  # 1-round-deferred exp/mask/mm2
                for kt2 in range(ktmax):
                    rel = max(128 * kt2, q0) - q0
                    s_t = ps_s.tile([128, 1024], F32, tag="s")
                    nc.tensor.matmul(
                        s_t[:, rel:512],
                        rslice(kT[pr], 0, kt2 * 128, (kt2 + 1) * 128),
                        rslice(qT[pr], 0, q0 + rel, q0 + 512),
                        start=True, stop=True,
                    )
                    nc.tensor.matmul(
                        s_t[:, 512 + rel:1024],
                        rslice(kT[pr], 64, kt2 * 128, (kt2 + 1) * 128),
                        rslice(qT[pr], 64, q0 + rel, q0 + 512),
                        start=True, stop=True,
                    )

                    def consume(kt2=kt2, rel=rel, s_t=s_t, ktmax=ktmax, qc=qc,
                                v3A=v3A, v3B=v3B, out_t=out_t, ndiag=ndiag):
                        diag = kt2 >= ndiag
                        last = kt2 == ktmax - 1
                        s3v = s_t[:].rearrange("p (x q) -> p x q", x=2)
                        ex = expp.tile([128, 1024], BF16, tag="ex")
                        e3 = ex[:].rearrange("p (x q) -> p x q", x=2)
                        if diag or kt2 >= exp_dve:
                            nc.scalar.activation(
                                e3[:, :, rel:], s3v[:, :, rel:], EXP,
                                scale=0.125
                            )
                        else:
                            e3i = ex[:].bitcast(I16).rearrange(
                                "p (x q) -> p x q", x=2)
                            nc.vector.tensor_scalar(
                                e3i[:, :, rel:], s3v[:, :, rel:],
                                FE_A, FE_B, MULT, ADD,
                            )
                        if diag:
                            nc.vector.tensor_mul(
                                e3[:, :, rel:rel + 128],
                                e3[:, :, rel:rel + 128],
                                tri_t[:].rearrange("p (x q) -> p x q", x=2),
                            )
                        for hf, v3 in ((0, v3A), (1, v3B)):
                            nc.tensor.matmul(
                                out_t[:, 512 * hf + rel:512 * hf + 512],
                                v3[:, kt2, :],
                                ex[:, 512 * hf + rel:512 * hf + 512],
                                start=(kt2 == 0), stop=last,
                            )

                    stage.append(consume)
                    if len(stage) > 1:
                        stage.pop(0)()
                while stage:
                    stage.pop(0)()

                # previous chunk's broadcast/normalize/store (its divide has
                # had a full chunk to finish -> no PE in-order stall)
                flush_norm()

                # drain accumulators promptly
                so = sop.tile([65, 1024], F32, tag="so")
                nc.vector.tensor_copy(so[:], out_t[:])

                def norm(qc=qc, so=so, hA=hA, hB=hB):
                    for hf, hh in ((0, hA), (1, hB)):
                        ob = obs[hh]
                        tr4 = ps_t.tile([128, 4 * 66], F32, tag="tr")
                        for j in range(kpc):
                            nc.tensor.transpose(
                                tr4[:, j * 66:j * 66 + 65],
                                so[:, hf * 512 + j * 128:
                                   hf * 512 + (j + 1) * 128],
                                id_t[:],
                            )
                        rc = rcp.tile([128, 4], F32, tag="rc")
                        for j in range(kpc):
                            nc.vector.reciprocal(
                                rc[:, j:j + 1], tr4[:, j * 66 + 64:j * 66 + 65]
                            )
                        for j in range(kpc):
                            jj = qc * kpc + j
                            nc.vector.tensor_scalar_mul(
                                ob[:, jj * DH:(jj + 1) * DH],
                                tr4[:, j * 66:j * 66 + DH], rc[:, j:j + 1]
                            )
                        c0 = qc * kpc * DH
                        nc.sync.dma_start(
                            o[hh][:, c0:c0 + kpc * DH], ob[:, c0:c0 + kpc * DH]
                        )

                pending_norm.append(norm)
            flush_norm()
        flush_norm()


# ---------------------------------------------------------------- host side


def _rope_tables_T(S):
    """Transposed tables cos|sin [128, 2S] for the stacked pair layout."""
    d = np.arange(DH, dtype=np.float32)
    div = np.float32(10000.0) ** ((d // 2 * 2).astype(np.float32) / np.float32(DH))
    pos = np.arange(S, dtype=np.float32)
    ang = pos[None, :] / div[:, None]          # (64, S)
    cosT = np.concatenate([np.cos(ang)] * 2, axis=0)  # (128, S)
    sinT = np.concatenate([np.sin(ang)] * 2, axis=0)
    return np.concatenate([cosT, sinT], axis=1).astype(ml_dtypes.bfloat16)


def host_inputs(qh, kh, vh, S):
    """Per-core input prep.  qh/kh/vh: (n_heads, S, DH) fp32."""
    n_heads = qh.shape[0]
    NT = S // 128
    npairs = n_heads // 2

    def pack(x):
        # (n_heads, S, DH) -> transposed (npairs, 128, S), then pack the
        # pair-swapped sign-folded shuffle alongside -> (npairs, 128, 2S)
        a = x.reshape(npairs, 2, S, DH).transpose(0, 1, 3, 2)  # (pr,2,DH,S)
        xT = np.ascontiguousarray(a.reshape(npairs, 128, S))
        sh = np.empty_like(xT)
        sh[:, 0::2] = -xT[:, 1::2]
        sh[:, 1::2] = xT[:, 0::2]
        return np.concatenate([xT, sh], axis=2)

    qq = pack(qh)
    kk = pack(kh)

    vt = vh.reshape(n_heads, NT, 128, DH).transpose(0, 2, 1, 3)  # (h,128,NT,DH)
    vext = np.concatenate(
        [vt, np.ones((n_heads, 128, NT, 1), np.float32)], axis=3
    ).astype(ml_dtypes.bfloat16)

    tri1 = np.triu(np.ones((128, 128), np.float32))
    tri = np.concatenate([tri1, tri1], axis=1).astype(ml_dtypes.bfloat16)

    bf = ml_dtypes.bfloat16
    return {
        "qqs": qq.astype(bf),
        "kks": kk.astype(bf),
        "vx": np.ascontiguousarray(vext.reshape(n_heads, 128, NT * 65)),
        "tt": _rope_tables_T(S),
        "tri": tri,
        "ident": np.eye(65, dtype=np.float32),
    }


_NC_CACHE = {}


def _get_nc():
    if "nc" not in _NC_CACHE:
        _NC_CACHE["nc"] = build_nc()
    return _NC_CACHE["nc"]


def kernel(q, k, v):
    q = np.asarray(q)
    k = np.asarray(k)
    v = np.asarray(v)
    nc = _get_nc()

    qh = q.reshape(B * H, S_FULL, DH)
    kh = k.reshape(B * H, S_FULL, DH)
    vh = v.reshape(B * H, S_FULL, DH)

    in_maps = []
    for c in range(N_CORES):
        sl = slice(c * HEADS_PER_CORE, (c + 1) * HEADS_PER_CORE)
        in_maps.append(host_inputs(qh[sl], kh[sl], vh[sl], S_FULL))

    res = run_bass_kernel_spmd(nc, in_maps, list(range(N_CORES)))

    NT = S_FULL // 128
    out = np.empty((B * H, S_FULL, DH), np.float32)
    for c in range(N_CORES):
        oc = np.asarray(res.results[c]["o"]).astype(np.float32)
        oc = oc.reshape(HEADS_PER_CORE, 128, NT, DH).transpose(0, 2, 1, 3)
        out[c * HEADS_PER_CORE:(c + 1) * HEADS_PER_CORE] = oc.reshape(
            HEADS_PER_CORE, S_FULL, DH
        )
    return out.reshape(B, S_FULL, H * DH)


# revision 17
# speedup vs baseline: 1.1631x; 1.1631x over previous
"""Multi-head self-attention (RoPE + causal softmax) Bass kernel for TRN2.

Problem: B=2, H=16, S=2048, D_HEAD=64, fp32 I/O.
Sharding: 32 head-instances (B*H) split 4-per-core across 8 NeuronCores;
no cross-device communication.

v3 design (per core, 4 heads = 2 stacked pairs):
  - Q,K ship host-pre-transposed bf16 with their pair-swapped sign-folded
    shuffles packed alongside (kks/qqs = [128, 2S]); cos|sin packed as one
    [128, 2S] table tensor; V (+ones column) as one 4-head tensor.  All
    loads via the two HWDGE rings (sync + scalar) as a few big DMAs.
  - RoPE on DVE: rot = k*cosT + kshuf*sinT, all-bf16 2x mode, emitted per
    1024-col half into separate half tiles so score matmuls start as soon
    as the first halves are rotated.
  - Scores per 128-row k-tile into [128, 2x512] PSUM (head A | head B),
    causally trimmed; head pair shares the PE via row-group concurrency.
  - exp(s/8): ScalarE exact exp for diagonal tiles + most others; a
    configurable number of old (large-context) k-tiles per chunk use a
    bf16 Schraudolph bit-trick on DVE (i16 = round(s*A+B) viewed as bf16).
  - attn@[V|1] accumulates outT [65, 2x512] over k-tiles; row 64 is the
    softmax denominator.
  - Normalization stays transposed (no PE transposes): copy PSUM->SBUF,
    GpSimd divide -> reciprocal row, PE ones-matmul broadcasts it across
    partitions, DVE multiply -> bf16 [64, q] output, DMA'd transposed;
    host un-transposes.  The PE broadcast is deferred one chunk so the
    in-order PE queue never waits on the divide.
  - Continuous PE warmup bridges the load/RoPE phase.
"""

import math

import numpy as np
import ml_dtypes

import concourse.bass as bass
import concourse.tile as tile
from concourse import bacc, mybir
from concourse.bass_utils import run_bass_kernel_spmd

F32 = mybir.dt.float32
BF16 = mybir.dt.bfloat16
I16 = mybir.dt.int16
EXP = mybir.ActivationFunctionType.Exp
MULT = mybir.AluOpType.mult
ADD = mybir.AluOpType.add
DIV = mybir.AluOpType.divide

B, H, S_FULL, DH = 2, 16, 2048, 64
N_CORES = 8
HEADS_PER_CORE = (B * H) // N_CORES  # 4

# Schraudolph fast-exp constants for bf16 (computing exp(s/8)):
# i16 = round(s * FE_A + FE_B); bits(i16) viewed as bf16 ~= exp(s/8).
FE_A = 128.0 / math.log(2.0) * 0.125
FE_B = 127.0 * 128.0 - 7.38

# Number of oldest non-diagonal k-tiles per chunk whose exp runs on DVE
# via the bit-trick (large-context rows only; diagonal tiles and chunk 0
# always use exact ScalarE exp).
EXP_DVE = 2
WARMUP_MMS = 10


# ---------------------------------------------------------------- device IR


def build_nc(n_heads=HEADS_PER_CORE, S=S_FULL, chunk=512, num_devices=N_CORES,
             exp_dve=None, warmup=None):
    NT = S // 128
    npairs = n_heads // 2

    nc = bacc.Bacc(
        "TRN2", target_bir_lowering=False, debug=False, num_devices=num_devices
    )

    qqs = nc.dram_tensor("qqs", [npairs, 128, 2 * S], BF16, kind="ExternalInput").ap()
    kks = nc.dram_tensor("kks", [npairs, 128, 2 * S], BF16, kind="ExternalInput").ap()
    vx = nc.dram_tensor("vx", [n_heads, 128, NT * 65], BF16, kind="ExternalInput").ap()
    tt = nc.dram_tensor("tt", [128, 2 * S], BF16, kind="ExternalInput").ap()
    tri = nc.dram_tensor("tri", [128, 256], BF16, kind="ExternalInput").ap()
    ident = nc.dram_tensor("ident", [65, 65], F32, kind="ExternalInput").ap()
    o = nc.dram_tensor("o", [n_heads, 128, (S // 128) * DH], BF16,
                       kind="ExternalOutput").ap()

    with tile.TileContext(nc) as tc:
        _body(nc, tc, qqs, kks, vx, tt, tri, ident, o,
              n_heads=n_heads, S=S, chunk=chunk,
              exp_dve=EXP_DVE if exp_dve is None else exp_dve,
              warmup=WARMUP_MMS if warmup is None else warmup)

    nc.compile()
    return nc


def _body(nc, tc, qqs, kks, vx, tt, tri, ident, o, *, n_heads, S, chunk,
          exp_dve, warmup):
    from contextlib import ExitStack

    assert chunk == 512
    NT = S // 128
    npairs = n_heads // 2
    nchunks = S // chunk
    kpc = chunk // 128
    HSZ = S // 2  # rope half size

    with ExitStack() as ctx:
        cpool = ctx.enter_context(tc.tile_pool(name="const", bufs=1))
        raw = ctx.enter_context(tc.tile_pool(name="raw", bufs=1))
        rot = ctx.enter_context(tc.tile_pool(name="rot", bufs=1))
        prep = ctx.enter_context(tc.tile_pool(name="prep", bufs=2))
        expp = ctx.enter_context(tc.tile_pool(name="expp", bufs=3))
        sop = ctx.enter_context(tc.tile_pool(name="sop", bufs=2))
        rcp = ctx.enter_context(tc.tile_pool(name="rcp", bufs=4))
        obuf = ctx.enter_context(tc.tile_pool(name="obuf", bufs=1))
        ps_s = ctx.enter_context(tc.tile_pool(name="ps_s", bufs=2, space="PSUM"))
        ps_o = ctx.enter_context(tc.tile_pool(name="ps_o", bufs=1, space="PSUM"))
        ps_t = ctx.enter_context(tc.tile_pool(name="ps_t", bufs=2, space="PSUM"))

        # ---- warmup seed + ACT exp-table preload
        wt = cpool.tile([128, 512], BF16, tag="wt")
        nc.vector.memset(wt[:], 0.25)
        id_t = cpool.tile([65, 65], F32, tag="id")
        dme = cpool.tile([128, 8], BF16, tag="dme")
        nc.scalar.activation(dme[:], wt[:, 0:8], EXP, scale=0.125)

        # ---- inputs: sync ring carries pair-0 (critical path), scalar ring
        # carries tables/V/pair-1.  Few big DMAs; no SWDGE.
        kqr = []  # (kks_tile, qqs_tile) per pair
        for p in range(npairs):
            kt_ = raw.tile([128, 2 * S], BF16, tag=f"kk{p}", name=f"kk{p}")
            qt_ = raw.tile([128, 2 * S], BF16, tag=f"qq{p}", name=f"qq{p}")
            kqr.append((kt_, qt_))
        tt_t = cpool.tile([128, 2 * S], BF16, tag="tt")
        vall = cpool.tile([128, n_heads * NT * 65], BF16, tag="vall")
        tri_t = cpool.tile([128, 256], BF16, tag="tri")

        for t_, d_ in ((kqr[0][0], kks[0]), (kqr[0][1], qqs[0])):
            t4 = t_[:].rearrange("p (g c) -> p g c", c=S // 2)
            d4 = d_.rearrange("p (g c) -> p g c", c=S // 2)
            nc.sync.dma_start(t4[:, 0::2, :], d4[:, 0::2, :])
            nc.sync.dma_start(t4[:, 1::2, :], d4[:, 1::2, :])
        nc.sync.dma_start(tri_t[:], tri[:])
        nc.sync.dma_start(id_t[:], ident[:])
        nc.scalar.dma_start(tt_t[:], tt[:])
        for h in range(n_heads):
            nc.scalar.dma_start(
                vall[:, h * NT * 65:(h + 1) * NT * 65], vx[h]
            )
        for p in range(1, npairs):
            nc.scalar.dma_start(kqr[p][0][:], kks[p])
            nc.scalar.dma_start(kqr[p][1][:], qqs[p])

        # ---- PE warmup bridge
        s_d = ps_s.tile([128, 1024], F32, tag="s")
        for _ in range(warmup):
            nc.tensor.matmul(s_d[:, 0:512], wt[0:64, 0:128], wt[0:64, 0:512],
                             start=True, stop=True)

        # ---- RoPE into per-half tiles: rot = nat*cos + shuf*sin (bf16 2x)
        cosv = tt_t[:, 0:S]
        sinv = tt_t[:, S:2 * S]
        kT = [[rot.tile([128, HSZ], BF16, tag=f"kT{p}{h}", name=f"kT{p}{h}")
               for h in range(2)] for p in range(npairs)]
        qT = [[rot.tile([128, HSZ], BF16, tag=f"qT{p}{h}", name=f"qT{p}{h}")
               for h in range(2)] for p in range(npairs)]
        chains = []
        for pr in range(npairs):
            chains.append((kqr[pr][0], kT[pr]))
            chains.append((kqr[pr][1], qT[pr]))
        for src, dsts in chains:
            for h in range(2):
                c0, c1 = h * HSZ, (h + 1) * HSZ
                t1 = prep.tile([128, HSZ], BF16, tag="t1")
                t2 = prep.tile([128, HSZ], BF16, tag="t2")
                nc.vector.tensor_mul(t1[:], src[:, c0:c1], cosv[:, c0:c1])
                nc.vector.tensor_mul(t2[:], src[:, S + c0:S + c1],
                                     sinv[:, c0:c1])
                nc.vector.tensor_add(dsts[h][:], t1[:], t2[:])

        def rslice(halves, base, lo, hi):
            # slice [lo:hi) columns out of the two half tiles, rows base:base+64
            h = lo // HSZ
            assert hi <= (h + 1) * HSZ
            return halves[h][base:base + 64, lo - h * HSZ:hi - h * HSZ]

        obs = [obuf.tile([128, NT * DH], BF16, tag=f"ob{h}", name=f"ob{h}")
               for h in range(n_heads)]

        # ---- main loop
        pending_norm = []

        def flush_norm():
            while pending_norm:
                pending_norm.pop(0)()

        for pr in range(npairs):
            hA, hB = 2 * pr, 2 * pr + 1
            v3A = vall[:, hA * NT * 65:(hA + 1) * NT * 65].rearrange(
                "p (t j) -> p t j", j=65)
            v3B = vall[:, hB * NT * 65:(hB + 1) * NT * 65].rearrange(
                "p (t j) -> p t j", j=65)
            for qc in range(nchunks):
                q0 = qc * chunk
                ktmax = (qc + 1) * kpc
                ndiag = qc * kpc
                out_t = ps_o.tile([65, 1024], F32, tag="out")

                stage = []  # 1-round-deferred exp/mask/mm2
                for kt2 in range(ktmax):
                    rel = max(128 * kt2, q0) - q0
                    s_t = ps_s.tile([128, 1024], F32, tag="s")
                    nc.tensor.matmul(
                        s_t[:, rel:512],
                        rslice(kT[pr], 0, kt2 * 128, (kt2 + 1) * 128),
                        rslice(qT[pr], 0, q0 + rel, q0 + 512),
                        start=True, stop=True,
                    )
                    nc.tensor.matmul(
                        s_t[:, 512 + rel:1024],
                        rslice(kT[pr], 64, kt2 * 128, (kt2 + 1) * 128),
                        rslice(qT[pr], 64, q0 + rel, q0 + 512),
                        start=True, stop=True,
                    )

                    def consume(kt2=kt2, rel=rel, s_t=s_t, ktmax=ktmax, qc=qc,
                                v3A=v3A, v3B=v3B, out_t=out_t, ndiag=ndiag):
                        diag = kt2 >= ndiag
                        last = kt2 == ktmax - 1
                        s3v = s_t[:].rearrange("p (x q) -> p x q", x=2)
                        ex = expp.tile([128, 1024], BF16, tag="ex")
                        e3 = ex[:].rearrange("p (x q) -> p x q", x=2)
                        if diag or kt2 >= exp_dve:
                            nc.scalar.activation(
                                e3[:, :, rel:], s3v[:, :, rel:], EXP,
                                scale=0.125
                            )
                        else:
                            assert rel == 0
                            nc.vector.tensor_scalar(
                                ex[:].bitcast(I16), s_t[:],
                                FE_A, FE_B, MULT, ADD,
                            )
                        if diag:
                            nc.vector.tensor_mul(
                                e3[:, :, rel:rel + 128],
                                e3[:, :, rel:rel + 128],
                                tri_t[:].rearrange("p (x q) -> p x q", x=2),
                            )
                        for hf, v3 in ((0, v3A), (1, v3B)):
                            nc.tensor.matmul(
                                out_t[:, 512 * hf + rel:512 * hf + 512],
                                v3[:, kt2, :],
                                ex[:, 512 * hf + rel:512 * hf + 512],
                                start=(kt2 == 0), stop=last,
                            )

                    stage.append(consume)
                    if len(stage) > 1:
                        stage.pop(0)()
                while stage:
                    stage.pop(0)()

                # previous chunk's broadcast/normalize/store (its divide has
                # had a full chunk to finish -> no PE in-order stall)
                flush_norm()

                # drain accumulators promptly
                so = sop.tile([65, 1024], F32, tag="so")
                nc.vector.tensor_copy(so[:], out_t[:])

                def norm(qc=qc, so=so, hA=hA, hB=hB):
                    for hf, hh in ((0, hA), (1, hB)):
                        ob = obs[hh]
                        tr4 = ps_t.tile([128, 4 * 66], F32, tag="tr")
                        for j in range(kpc):
                            nc.tensor.transpose(
                                tr4[:, j * 66:j * 66 + 65],
                                so[:, hf * 512 + j * 128:
                                   hf * 512 + (j + 1) * 128],
                                id_t[:],
                            )
                        rc = rcp.tile([128, 4], F32, tag="rc")
                        t3 = tr4[:].rearrange("p (j c) -> p j c", c=66)
                        nc.vector.reciprocal(rc[:], t3[:, :, 64])
                        for j in range(kpc):
                            jj = qc * kpc + j
                            nc.vector.tensor_scalar_mul(
                                ob[:, jj * DH:(jj + 1) * DH],
                                tr4[:, j * 66:j * 66 + DH], rc[:, j:j + 1]
                            )
                        c0 = qc * kpc * DH
                        nc.sync.dma_start(
                            o[hh][:, c0:c0 + kpc * DH], ob[:, c0:c0 + kpc * DH]
                        )

                if pr == npairs - 1 and qc == nchunks - 1:
                    norm()
                else:
                    pending_norm.append(norm)
            flush_norm()
        flush_norm()


# ---------------------------------------------------------------- host side


def _rope_tables_T(S):
    """Transposed tables cos|sin [128, 2S] for the stacked pair layout."""
    d = np.arange(DH, dtype=np.float32)
    div = np.float32(10000.0) ** ((d // 2 * 2).astype(np.float32) / np.float32(DH))
    pos = np.arange(S, dtype=np.float32)
    ang = pos[None, :] / div[:, None]          # (64, S)
    cosT = np.concatenate([np.cos(ang)] * 2, axis=0)  # (128, S)
    sinT = np.concatenate([np.sin(ang)] * 2, axis=0)
    return np.concatenate([cosT, sinT], axis=1).astype(ml_dtypes.bfloat16)


def host_inputs(qh, kh, vh, S):
    """Per-core input prep.  qh/kh/vh: (n_heads, S, DH) fp32."""
    n_heads = qh.shape[0]
    NT = S // 128
    npairs = n_heads // 2

    def pack(x):
        # (n_heads, S, DH) -> transposed (npairs, 128, S), then pack the
        # pair-swapped sign-folded shuffle alongside -> (npairs, 128, 2S)
        a = x.reshape(npairs, 2, S, DH).transpose(0, 1, 3, 2)  # (pr,2,DH,S)
        xT = np.ascontiguousarray(a.reshape(npairs, 128, S))
        sh = np.empty_like(xT)
        sh[:, 0::2] = -xT[:, 1::2]
        sh[:, 1::2] = xT[:, 0::2]
        return np.concatenate([xT, sh], axis=2)

    qq = pack(qh)
    kk = pack(kh)

    vt = vh.reshape(n_heads, NT, 128, DH).transpose(0, 2, 1, 3)  # (h,128,NT,DH)
    vext = np.concatenate(
        [vt, np.ones((n_heads, 128, NT, 1), np.float32)], axis=3
    ).astype(ml_dtypes.bfloat16)

    tri1 = np.triu(np.ones((128, 128), np.float32))
    tri = np.concatenate([tri1, tri1], axis=1).astype(ml_dtypes.bfloat16)

    bf = ml_dtypes.bfloat16
    return {
        "qqs": qq.astype(bf),
        "kks": kk.astype(bf),
        "vx": np.ascontiguousarray(vext.reshape(n_heads, 128, NT * 65)),
        "tt": _rope_tables_T(S),
        "tri": tri,
        "ident": np.eye(65, dtype=np.float32),
    }


_NC_CACHE = {}


def _get_nc():
    if "nc" not in _NC_CACHE:
        _NC_CACHE["nc"] = build_nc()
    return _NC_CACHE["nc"]


def kernel(q, k, v):
    q = np.asarray(q)
    k = np.asarray(k)
    v = np.asarray(v)
    nc = _get_nc()

    qh = q.reshape(B * H, S_FULL, DH)
    kh = k.reshape(B * H, S_FULL, DH)
    vh = v.reshape(B * H, S_FULL, DH)

    in_maps = []
    for c in range(N_CORES):
        sl = slice(c * HEADS_PER_CORE, (c + 1) * HEADS_PER_CORE)
        in_maps.append(host_inputs(qh[sl], kh[sl], vh[sl], S_FULL))

    res = run_bass_kernel_spmd(nc, in_maps, list(range(N_CORES)))

    NT = S_FULL // 128
    out = np.empty((B * H, S_FULL, DH), np.float32)
    for c in range(N_CORES):
        oc = np.asarray(res.results[c]["o"]).astype(np.float32)
        oc = oc.reshape(HEADS_PER_CORE, 128, NT, DH).transpose(0, 2, 1, 3)
        out[c * HEADS_PER_CORE:(c + 1) * HEADS_PER_CORE] = oc.reshape(
            HEADS_PER_CORE, S_FULL, DH
        )
    return out.reshape(B, S_FULL, H * DH)
